# revision 1
# baseline (speedup 1.0000x reference)
"""DGCNN encoder Bass kernel for Trainium2, data-parallel over batch on 8 cores.

Per core (one sample, x: (2048, 3)):
  4 EdgeConv layers + final 1x1 conv + global max/avg pool -> (2048,) output row.

Key algebraic restructuring (exact, since the BN scale gamma*rsqrt(var+eps) > 0
and leaky-relu is monotone):
  edgeconv(x)[n] = bnlrelu( max_{j in knn(n)} (Wa @ x_j)  +  (Wb - Wa) @ x_n )
with W = [Wa | Wb] acting on [x_j - x_n ; x_n].  This removes the k=20 factor
from all matmuls; only the top-20 selection and a row-gather + max remain.

kNN selection per 128-row block: score[n, j] = 2 x_n.x_j - |x_j|^2 (row-shifted
negated squared distance, same per-row order) via PE matmuls, then three
max8 / max_index / match_replace rounds on DVE for the exact top-20 set.

Neighbor gather: indices are bounced through DRAM into the 16-partition-wrapped
layout the GPSIMD indirect_copy expects (same index list for every partition
group), gathering u^T = (Wa @ x)^T rows; max over k=20 via a tensor_reduce.

Static layout transforms (x -> x^T, W -> Wa^T / (Wb-Wa)^T, W5^T chunks,
bn -> bn^T) happen host-side in numpy: element-granularity strided DRAM DMAs
abort the NRT on this stack, and contiguous feeds make them unnecessary.
"""
import sys
sys.path.insert(0, '/opt/trn_rl_repo')

import numpy as np
import concourse.bass as bass
import concourse.bacc as bacc
import concourse.tile as tile
from concourse import mybir

f32 = mybir.dt.float32
u16 = mybir.dt.uint16
bf16 = mybir.dt.bfloat16
Alu = mybir.AluOpType
Act = mybir.ActivationFunctionType

N = 2048
NBLK = N // 128
KNN = 20
NEG_SLOPE = 0.2
BN_EPS = 1e-5
NEG_BIG = -1e30

# (C_in, O) per edge-conv layer
LAYERS = [(3, 64), (64, 64), (64, 128), (128, 256)]
# W5^T host-side chunks aligned to the xcat source tiles
W5_CHUNKS = [(0, 64), (64, 128), (128, 256), (256, 384), (384, 512)]


def _ceil(a, b):
    return (a + b - 1) // b


def build_dgcnn(nc):
    """Emit the full per-core DGCNN program into nc."""
    xT_d = nc.dram_tensor("xT", [3, N], f32, kind="ExternalInput")
    WaT_d = [nc.dram_tensor(f"WaT{l+1}", [c, o], f32, kind="ExternalInput")
             for l, (c, o) in enumerate(LAYERS)]
    WdT_d = [nc.dram_tensor(f"WdT{l+1}", [c, o], f32, kind="ExternalInput")
             for l, (c, o) in enumerate(LAYERS)]
    W5T_d = [nc.dram_tensor(f"W5T_{lo}", [hi - lo, 1024], f32,
                            kind="ExternalInput") for lo, hi in W5_CHUNKS]
    bnT_d = [nc.dram_tensor(f"bnT{l+1}", [o, 4], f32, kind="ExternalInput")
             for l, (c, o) in enumerate(LAYERS)]
    bn5T_d = nc.dram_tensor("bn5T", [1024, 4], f32, kind="ExternalInput")
    out_d = nc.dram_tensor("out", [2048], f32, kind="ExternalOutput")
    # DRAM bounce for the index wrap-relayout, n-major: list[n*20+k] = idx[n,k]
    list_d = nc.dram_tensor("idxlist", [NBLK * 128 * KNN], u16, kind="Internal")

    with tile.TileContext(nc) as tc:
        from contextlib import ExitStack
        ctx = ExitStack()
        with ctx:
            persist = ctx.enter_context(tc.tile_pool(name="persist", bufs=1))
            work = ctx.enter_context(tc.tile_pool(name="work", bufs=2))

            onesC = persist.tile([128, 1], f32, tag="onesC")
            nc.vector.memset(onesC, 1.0)
            ones1 = persist.tile([1, 128], f32, tag="ones1")
            nc.vector.memset(ones1, 1.0)
            eps_t = persist.tile([128, 1], f32, tag="eps")
            nc.vector.memset(eps_t, BN_EPS)

            # ---- bn param prep: (C, 4) rows [gamma, beta, mean, var] ->
            #      scale (C,1), bias (C,1) tiles per 128-channel chunk
            def prep_bn(bn_dram, channels, name):
                scales, biases = [], []
                for t in range(_ceil(channels, 128)):
                    p = min(128, channels - t * 128)
                    raw = work.tile([128, 4], f32, tag="bnraw")
                    src = bass.AP(tensor=bn_dram, offset=t * 128 * 4,
                                  ap=[[4, p], [1, 4]])
                    nc.gpsimd.dma_start(out=raw[:p, :], in_=src)
                    s_t = persist.tile([128, 1], f32, tag=f"{name}_s{t}")
                    b_t = persist.tile([128, 1], f32, tag=f"{name}_b{t}")
                    tmp = work.tile([128, 1], f32, tag="bntmp")
                    nc.scalar.activation(tmp[:p], raw[:p, 3:4], Act.Sqrt,
                                         bias=eps_t[:p], scale=1.0)
                    nc.vector.reciprocal(tmp[:p], tmp[:p])
                    nc.vector.tensor_mul(s_t[:p], raw[:p, 0:1], tmp[:p])
                    nc.vector.tensor_mul(tmp[:p], raw[:p, 2:3], s_t[:p])
                    nc.vector.tensor_sub(b_t[:p], raw[:p, 1:2], tmp[:p])
                    scales.append(s_t)
                    biases.append(b_t)
                return scales, biases

            bn_sb = [prep_bn(bnT_d[l], o, f"bn{l}")
                     for l, (c, o) in enumerate(LAYERS)]
            bn5_s, bn5_b = prep_bn(bn5T_d, 1024, "bn5")

            # ---- weights (already transposed host-side)
            WaT, WdT = [], []
            for l, (c, o) in enumerate(LAYERS):
                wa = persist.tile([max(c, 16), o], f32, tag=f"WaT{l}",
                                  name=f"WaT{l}")
                nc.gpsimd.dma_start(out=wa[:c, :], in_=WaT_d[l][:, :])
                wd = persist.tile([max(c, 16), o], f32, tag=f"WdT{l}",
                                  name=f"WdT{l}")
                nc.gpsimd.dma_start(out=wd[:c, :], in_=WdT_d[l][:, :])
                WaT.append(wa)
                WdT.append(wd)
            W5T = []
            for i, (lo, hi) in enumerate(W5_CHUNKS):
                t5 = persist.tile([max(hi - lo, 16), 1024], bf16,
                                  tag=f"W5T_{lo}", name=f"W5T_{lo}")
                nc.gpsimd.dma_start(out=t5[:hi - lo, :], in_=W5T_d[i][:, :])
                W5T.append(t5)

            x0T = persist.tile([16, N], f32, tag="x0T")
            nc.gpsimd.dma_start(out=x0T[:3, :], in_=xT_d[:, :])

            # ---- edge conv layers
            def edge_conv(l, c, o, xT, out_tag):
                notile = _ceil(o, 128)
                aug = c + 1 <= 65
                with ExitStack() as lx:
                    lwork = lx.enter_context(
                        tc.tile_pool(name=f"lwork{l}", bufs=2))
                    prep_ps = tc.tile_pool(name=f"prep_ps{l}", bufs=2,
                                           space="PSUM")
                    with prep_ps as pp:
                        sq = lwork.tile([max(c, 16), N], f32, tag="sq", bufs=1)
                        nc.scalar.square(sq[:c, :], xT[:c, :])
                        if aug:
                            lhs_sc = lwork.tile([c + 1, N], f32, tag="lhs_sc",
                                                bufs=1)
                            rhs_sc = lwork.tile([c + 1, N], f32, tag="rhs_sc",
                                                bufs=1)
                            aligned = (c % 32) == 0
                            if aligned:
                                nc.vector.tensor_scalar_mul(lhs_sc[:c, :],
                                                            xT[:c, :], 2.0)
                                nc.vector.memset(lhs_sc[c:c + 1, :], 1.0)
                                nc.scalar.copy(rhs_sc[:c, :], xT[:c, :])
                                negsq_dst = rhs_sc[c:c + 1, :]
                            else:
                                # engine APs must start 32-aligned: fill the
                                # ones row via full-height memset; negsq goes
                                # through a base-0 tile + contiguous SBUF DMA
                                nc.vector.memset(lhs_sc[:c + 1, :], 1.0)
                                nc.vector.tensor_scalar_mul(lhs_sc[:c, :],
                                                            xT[:c, :], 2.0)
                                nc.scalar.copy(rhs_sc[:c, :], xT[:c, :])
                                negsq = lwork.tile([1, N], f32, tag="negsq",
                                                   bufs=1)
                                negsq_dst = negsq[:, :]
                        else:
                            lhs_sc = lwork.tile([c, N], f32, tag="lhs_sc",
                                                bufs=1)
                            rhs_sc = xT
                            nc.vector.tensor_scalar_mul(lhs_sc[:c, :],
                                                        xT[:c, :], 2.0)
                            negsq = lwork.tile([1, N], f32, tag="negsq", bufs=1)
                            negsq_dst = negsq[:, :]
                        for ch in range(4):
                            cs = slice(ch * 512, (ch + 1) * 512)
                            nps = pp.tile([1, 512], f32, tag="negsq_ps")
                            nc.tensor.matmul(nps, lhsT=onesC[:c, :],
                                             rhs=sq[:c, cs],
                                             start=True, stop=True)
                            nc.scalar.mul(negsq_dst[:, cs], nps, -1.0)
                        if aug and not aligned:
                            nc.gpsimd.dma_start(out=rhs_sc[c:c + 1, :],
                                                in_=negsq[:, :])

                        # --- u, v
                        u_sb = [lwork.tile([128, N], f32, tag=f"u_sb{t}",
                                           name=f"u_sb{t}", bufs=1)
                                for t in range(notile)]
                        v_sb = [lwork.tile([128, N], f32, tag=f"v_sb{t}",
                                           name=f"v_sb{t}", bufs=1)
                                for t in range(notile)]
                        rem = o - (notile - 1) * 128
                        if rem < 128:
                            nc.vector.memset(u_sb[notile - 1][rem:, :], 0.0)
                        for t in range(notile):
                            op = min(128, o - t * 128)
                            osl = slice(t * 128, t * 128 + op)
                            for ch in range(4):
                                cs = slice(ch * 512, (ch + 1) * 512)
                                ups = pp.tile([128, 512], f32, tag="u_ps")
                                nc.tensor.matmul(ups[:op, :],
                                                 lhsT=WaT[l][:c, osl],
                                                 rhs=xT[:c, cs],
                                                 start=True, stop=True)
                                nc.scalar.copy(u_sb[t][:op, cs], ups[:op, :])
                                vps = pp.tile([128, 512], f32, tag="v_ps")
                                nc.tensor.matmul(vps[:op, :],
                                                 lhsT=WdT[l][:c, osl],
                                                 rhs=xT[:c, cs],
                                                 start=True, stop=True)
                                nc.scalar.copy(v_sb[t][:op, cs], vps[:op, :])

                    # --- per-block: score -> top-20 -> gather -> max -> bn+lrelu
                    xout = [persist.tile([128, N], f32, tag=f"{out_tag}_{t}",
                                         name=f"{out_tag}_{t}")
                            for t in range(notile)]
                    with tc.tile_pool(name=f"sc_ps{l}", bufs=2,
                                      space="PSUM") as sp:
                        for b in range(NBLK):
                            bsl = slice(b * 128, (b + 1) * 128)
                            scps = sp.tile([128, N], f32, tag="scps")
                            for ch in range(4):
                                cs = slice(ch * 512, (ch + 1) * 512)
                                if aug:
                                    nc.tensor.matmul(scps[:, cs],
                                                     lhsT=lhs_sc[:c + 1, bsl],
                                                     rhs=rhs_sc[:c + 1, cs],
                                                     start=True, stop=True)
                                else:
                                    nc.tensor.matmul(scps[:, cs],
                                                     lhsT=lhs_sc[:c, bsl],
                                                     rhs=rhs_sc[:c, cs],
                                                     start=True, stop=False)
                                    nc.tensor.matmul(scps[:, cs], lhsT=ones1,
                                                     rhs=negsq[:, cs],
                                                     start=False, stop=True)
                            sc = lwork.tile([128, N], f32, tag="sc")
                            nc.scalar.copy(sc, scps)
                            idxb = lwork.tile([128, KNN], u16, tag="idxb")
                            vals = lwork.tile([128, 8], f32, tag="vals")
                            idx3 = lwork.tile([128, 8], u16, tag="idx3")
                            nc.vector.max(vals, sc)
                            nc.vector.max_index(idxb[:, 0:8], vals, sc)
                            nc.vector.match_replace(sc, vals, sc, NEG_BIG)
                            nc.vector.max(vals, sc)
                            nc.vector.max_index(idxb[:, 8:16], vals, sc)
                            nc.vector.match_replace(sc, vals, sc, NEG_BIG)
                            nc.vector.max(vals, sc)
                            nc.vector.max_index(idx3, vals, sc)
                            nc.vector.tensor_copy(idxb[:, 16:20], idx3[:, 0:4])

                            # n-major contiguous store: list[n*20+k] = idxb[n,k]
                            dst1 = bass.AP(tensor=list_d, offset=b * 2560,
                                           ap=[[KNN, 128], [1, KNN]])
                            nc.gpsimd.dma_start(out=dst1, in_=idxb[:, :])
                            # wrap read: wrapped[16g+p, s] = list[s*16+p]
                            wrapped = lwork.tile([128, 160], u16, tag="wrapped")
                            for g in range(8):
                                src2 = bass.AP(tensor=list_d, offset=b * 2560,
                                               ap=[[1, 16], [16, 160]])
                                nc.gpsimd.dma_start(
                                    out=wrapped[g * 16:(g + 1) * 16, :],
                                    in_=src2)
                            for t in range(notile):
                                op = min(128, o - t * 128)
                                gath = lwork.tile([128, 2560], f32, tag="gath")
                                # walrus caps indirect_copy at 1024 indices
                                for lo in range(0, 2560, 1024):
                                    hi = min(lo + 1024, 2560)
                                    nc.gpsimd.indirect_copy(
                                        gath[:, lo:hi], u_sb[t],
                                        wrapped[:, lo // 16:hi // 16], True)
                                m_sb = lwork.tile([128, 128], f32, tag="m_sb")
                                nc.vector.tensor_reduce(
                                    m_sb,
                                    gath.rearrange("p (n k) -> p n k", k=KNN),
                                    axis=mybir.AxisListType.X, op=Alu.max)
                                y = lwork.tile([128, 128], f32, tag="yb")
                                nc.vector.tensor_add(y[:op, :], m_sb[:op, :],
                                                     v_sb[t][:op, bsl])
                                ybn = lwork.tile([128, 128], f32, tag="ybn")
                                nc.scalar.activation(ybn[:op, :], y[:op, :],
                                                     Act.Identity,
                                                     bias=bn_sb[l][1][t][:op],
                                                     scale=bn_sb[l][0][t][:op])
                                nc.vector.scalar_tensor_tensor(
                                    xout[t][:op, bsl], ybn[:op, :], NEG_SLOPE,
                                    ybn[:op, :], op0=Alu.mult, op1=Alu.max)
                return xout

            x1 = edge_conv(0, 3, 64, x0T, "x1")
            x2 = edge_conv(1, 64, 64, x1[0], "x2")
            x3 = edge_conv(2, 64, 128, x2[0], "x3")
            x4 = edge_conv(3, 128, 256, x3[0], "x4")

            # ---- final 1x1 conv (W5, bf16) + BN + lrelu + global max/avg pool
            xcat_parts = [(x1[0], 64), (x2[0], 64), (x3[0], 128),
                          (x4[0], 128), (x4[1], 128)]
            inv_n = 1.0 / float(N)
            with tc.tile_pool(name="f_ps", bufs=4, space="PSUM") as fp, \
                 tc.tile_pool(name="fwork", bufs=2) as fw, \
                 tc.tile_pool(name="fb16", bufs=1) as fb:
                xcb = []
                for i, (xp, ck) in enumerate(xcat_parts):
                    xtile = fb.tile([max(ck, 16), N], bf16, tag=f"xcb{i}",
                                    name=f"xcb{i}")
                    nc.vector.tensor_copy(xtile[:ck, :], xp[:ck, :])
                    xcb.append(xtile)
                for ot in range(8):
                    osl = slice(ot * 128, (ot + 1) * 128)
                    sums = fw.tile([128, 4], f32, tag="sums")
                    gmax = fw.tile([128, 512], f32, tag="gmax512")
                    for chn in range(4):
                        cs = slice(chn * 512, (chn + 1) * 512)
                        fps = fp.tile([128, 512], f32, tag="fps")
                        for i, (xp, ck) in enumerate(xcat_parts):
                            nc.tensor.matmul(fps, lhsT=W5T[i][:ck, osl],
                                             rhs=xcb[i][:ck, cs],
                                             start=(i == 0), stop=(i == 4))
                        ybn = fw.tile([128, 512], f32, tag="fybn")
                        nc.scalar.activation(ybn, fps, Act.Identity,
                                             bias=bn5_b[ot], scale=bn5_s[ot])
                        feat = fw.tile([128, 512], f32, tag="feat")
                        nc.vector.scalar_tensor_tensor(
                            feat, ybn, NEG_SLOPE, ybn,
                            op0=Alu.mult, op1=Alu.max,
                            accum_out=sums[:, chn:chn + 1])
                        if chn == 0:
                            nc.vector.tensor_copy(gmax, feat)
                        else:
                            nc.vector.tensor_max(gmax, gmax, feat)
                    gm = fw.tile([128, 1], f32, tag="gm")
                    nc.vector.tensor_reduce(gm, gmax, axis=mybir.AxisListType.X,
                                            op=Alu.max)
                    ga = fw.tile([128, 1], f32, tag="ga")
                    nc.vector.tensor_reduce(ga, sums, axis=mybir.AxisListType.X,
                                            op=Alu.add)
                    nc.vector.tensor_scalar_mul(ga, ga, inv_n)
                    nc.gpsimd.dma_start(
                        out=bass.AP(tensor=out_d, offset=ot * 128,
                                    ap=[[1, 128]]),
                        in_=gm[:, :])
                    nc.gpsimd.dma_start(
                        out=bass.AP(tensor=out_d, offset=1024 + ot * 128,
                                    ap=[[1, 128]]),
                        in_=ga[:, :])


def host_prepare(inputs):
    """Full inputs -> per-core input maps (host-side layout transforms)."""
    x = np.asarray(inputs["x"], dtype=np.float32)
    B = x.shape[0]
    shared = {}
    for l, (c, o) in enumerate(LAYERS):
        W = np.asarray(inputs[f"W{l+1}"], dtype=np.float32)
        Wa = W[:, :c]
        Wd = W[:, c:] - Wa
        shared[f"WaT{l+1}"] = np.ascontiguousarray(Wa.T)
        shared[f"WdT{l+1}"] = np.ascontiguousarray(Wd.T)
        bn = np.asarray(inputs[f"bn{l+1}"], dtype=np.float32)
        shared[f"bnT{l+1}"] = np.ascontiguousarray(bn.T)
    W5 = np.asarray(inputs["W5"], dtype=np.float32)
    for lo, hi in W5_CHUNKS:
        shared[f"W5T_{lo}"] = np.ascontiguousarray(W5[:, lo:hi].T)
    shared["bn5T"] = np.ascontiguousarray(
        np.asarray(inputs["bn5"], dtype=np.float32).T)
    return [dict(shared, xT=np.ascontiguousarray(x[b].T)) for b in range(B)]


_CACHED = {}


def _get_compiled():
    if "nc" not in _CACHED:
        nc = bacc.Bacc("TRN2", target_bir_lowering=False, debug=False,
                       num_devices=8)
        build_dgcnn(nc)
        nc.compile()
        _CACHED["nc"] = nc
    return _CACHED["nc"]


def kernel(**inputs):
    from concourse.bass_utils import run_bass_kernel_spmd
    nc = _get_compiled()
    in_maps = host_prepare(inputs)
    res = run_bass_kernel_spmd(nc, in_maps, list(range(len(in_maps))))
    out = np.stack([np.asarray(res.results[b]["out"]).reshape(-1)
                    for b in range(len(in_maps))], axis=0)
    return out.astype(np.float32)



# revision 5
# speedup vs baseline: 1.1961x; 1.1961x over previous
"""DGCNN encoder Bass kernel for Trainium2, data-parallel over batch on 8 cores.

Per core (one sample, x: (2048, 3)):
  4 EdgeConv layers + final 1x1 conv + global max/avg pool -> (2048,) output row.

Restructure (exact): edgeconv(x)[n] = lrelu( max_{j in knn(n)} (s*Wa @ x_j)
  + (s*(Wb-Wa)) @ x_n + bias ) with the positive BN scale s folded into the
weights host-side (max commutes with positive scaling).

v2 layout: kNN selection unchanged (exact fp32 max8/max_index/match_replace
rounds on DVE). The neighbor aggregation is restructured as a DRAM row-gather:
uT rows (node-major, bn-scaled) are written per block to DRAM, then one
dma_gather per block fetches the 20 neighbor rows of each node into
[128 nodes, 20, o]; the k-max reduce runs on GPSIMD, bias+lrelu on DVE, and a
PE transpose brings the result back to the [channel, node] layout the next
layer consumes. All plain DMAs are issued from the SP/ACT engines (HWDGE)
instead of GPSIMD to avoid the ~1us SWDGE descriptor-generation tax per call.
"""
import sys
sys.path.insert(0, '/opt/trn_rl_repo')

import numpy as np
import concourse.bass as bass
import concourse.bacc as bacc
import concourse.tile as tile
from concourse import mybir

f32 = mybir.dt.float32
u16 = mybir.dt.uint16
i16 = mybir.dt.int16
bf16 = mybir.dt.bfloat16
Alu = mybir.AluOpType
Act = mybir.ActivationFunctionType

N = 2048
NBLK = N // 128
KNN = 20
NEG_SLOPE = 0.2
BN_EPS = 1e-5
NEG_BIG = -1e30

# (C_in, O) per edge-conv layer
LAYERS = [(3, 64), (64, 64), (64, 128), (128, 256)]
# W5^T host-side chunks aligned to the xcat source tiles
W5_CHUNKS = [(0, 64), (64, 128), (128, 256), (256, 384), (384, 512)]


def _ceil(a, b):
    return (a + b - 1) // b


def build_dgcnn(nc):
    """Emit the full per-core DGCNN program into nc."""
    xT_d = nc.dram_tensor("xT", [3, N], f32, kind="ExternalInput")
    WaT_d = [nc.dram_tensor(f"WaT{l+1}", [c, o], f32, kind="ExternalInput")
             for l, (c, o) in enumerate(LAYERS)]
    WdT_d = [nc.dram_tensor(f"WdT{l+1}", [c, o], f32, kind="ExternalInput")
             for l, (c, o) in enumerate(LAYERS)]
    bias_d = [nc.dram_tensor(f"biasv{l+1}", [o], f32, kind="ExternalInput")
              for l, (c, o) in enumerate(LAYERS)]
    W5T_d = [nc.dram_tensor(f"W5T_{lo}", [hi - lo, 1024], f32,
                            kind="ExternalInput") for lo, hi in W5_CHUNKS]
    bn5T_d = nc.dram_tensor("bn5T", [1024, 4], f32, kind="ExternalInput")
    ident_d = nc.dram_tensor("ident", [128, 128], f32, kind="ExternalInput")
    out_d = nc.dram_tensor("out", [2048], f32, kind="ExternalOutput")
    # per-layer DRAM bounces: j-major neighbor index list and node-major uT rows
    list_d = [nc.dram_tensor(f"idxlist{l}", [NBLK * 128 * KNN], u16,
                             kind="Internal") for l in range(4)]
    uT_d = [nc.dram_tensor(f"uT{l}", [N * o], f32, kind="Internal")
            for l, (c, o) in enumerate(LAYERS)]

    with tile.TileContext(nc) as tc:
        from contextlib import ExitStack
        ctx = ExitStack()
        with ctx:
            persist = ctx.enter_context(tc.tile_pool(name="persist", bufs=1))
            work = ctx.enter_context(tc.tile_pool(name="work", bufs=2))

            onesC = persist.tile([128, 1], f32, tag="onesC")
            nc.vector.memset(onesC, 1.0)
            ones1 = persist.tile([1, 128], f32, tag="ones1")
            nc.vector.memset(ones1, 1.0)
            eps_t = persist.tile([128, 1], f32, tag="eps")
            nc.vector.memset(eps_t, BN_EPS)
            ident = persist.tile([128, 128], f32, tag="ident")
            nc.sync.dma_start(out=ident, in_=ident_d[:, :])

            # ---- bn5 param prep: (C, 4) rows [gamma, beta, mean, var] ->
            #      scale (C,1), bias (C,1) tiles per 128-channel chunk
            def prep_bn(bn_dram, channels, name):
                scales, biases = [], []
                for t in range(_ceil(channels, 128)):
                    p = min(128, channels - t * 128)
                    raw = work.tile([128, 4], f32, tag="bnraw")
                    src = bass.AP(tensor=bn_dram, offset=t * 128 * 4,
                                  ap=[[4, p], [1, 4]])
                    nc.sync.dma_start(out=raw[:p, :], in_=src)
                    s_t = persist.tile([128, 1], f32, tag=f"{name}_s{t}")
                    b_t = persist.tile([128, 1], f32, tag=f"{name}_b{t}")
                    tmp = work.tile([128, 1], f32, tag="bntmp")
                    nc.scalar.activation(tmp[:p], raw[:p, 3:4], Act.Sqrt,
                                         bias=eps_t[:p], scale=1.0)
                    nc.vector.reciprocal(tmp[:p], tmp[:p])
                    nc.vector.tensor_mul(s_t[:p], raw[:p, 0:1], tmp[:p])
                    nc.vector.tensor_mul(tmp[:p], raw[:p, 2:3], s_t[:p])
                    nc.vector.tensor_sub(b_t[:p], raw[:p, 1:2], tmp[:p])
                    scales.append(s_t)
                    biases.append(b_t)
                return scales, biases

            bn5_s, bn5_b = prep_bn(bn5T_d, 1024, "bn5")

            # ---- weights (transposed + bn-scaled host-side)
            WaT, WdT, bias_bc = [], [], []
            for l, (c, o) in enumerate(LAYERS):
                wa = persist.tile([max(c, 16), o], f32, tag=f"WaT{l}",
                                  name=f"WaT{l}")
                nc.sync.dma_start(out=wa[:c, :], in_=WaT_d[l][:, :])
                wd = persist.tile([max(c, 16), o], f32, tag=f"WdT{l}",
                                  name=f"WdT{l}")
                nc.sync.dma_start(out=wd[:c, :], in_=WdT_d[l][:, :])
                WaT.append(wa)
                WdT.append(wd)
                # bias broadcast tile [128, o] (replicated rows)
                bb = persist.tile([128, o], f32, tag=f"biasbc{l}",
                                  name=f"biasbc{l}")
                nc.sync.dma_start(
                    out=bb,
                    in_=bass.AP(tensor=bias_d[l], offset=0,
                                ap=[[0, 128], [1, o]]))
                bias_bc.append(bb)
            W5T = []
            for i, (lo, hi) in enumerate(W5_CHUNKS):
                t5 = persist.tile([max(hi - lo, 16), 1024], bf16,
                                  tag=f"W5T_{lo}", name=f"W5T_{lo}")
                nc.gpsimd.dma_start(out=t5[:hi - lo, :], in_=W5T_d[i][:, :])
                W5T.append(t5)

            x0T = persist.tile([16, N], f32, tag="x0T")
            nc.sync.dma_start(out=x0T[:3, :], in_=xT_d[:, :])

            # ---- edge conv layers
            def edge_conv(l, c, o, xT, out_tag):
                notile = _ceil(o, 128)
                aug = c + 1 <= 65
                with ExitStack() as lx:
                    lwork = lx.enter_context(
                        tc.tile_pool(name=f"lwork{l}", bufs=2))
                    prep_ps = tc.tile_pool(name=f"prep_ps{l}", bufs=2,
                                           space="PSUM")
                    with prep_ps as pp:
                        sq = lwork.tile([max(c, 16), N], f32, tag="sq", bufs=1)
                        nc.scalar.square(sq[:c, :], xT[:c, :])
                        if aug:
                            lhs_sc = lwork.tile([c + 1, N], f32, tag="lhs_sc",
                                                bufs=1)
                            rhs_sc = lwork.tile([c + 1, N], f32, tag="rhs_sc",
                                                bufs=1)
                            aligned = (c % 32) == 0
                            if aligned:
                                nc.vector.tensor_scalar_mul(lhs_sc[:c, :],
                                                            xT[:c, :], 2.0)
                                nc.vector.memset(lhs_sc[c:c + 1, :], 1.0)
                                nc.scalar.copy(rhs_sc[:c, :], xT[:c, :])
                                negsq_dst = rhs_sc[c:c + 1, :]
                            else:
                                # engine APs must start 32-aligned: fill the
                                # ones row via full-height memset; negsq goes
                                # through a base-0 tile + contiguous SBUF DMA
                                nc.vector.memset(lhs_sc[:c + 1, :], 1.0)
                                nc.vector.tensor_scalar_mul(lhs_sc[:c, :],
                                                            xT[:c, :], 2.0)
                                nc.scalar.copy(rhs_sc[:c, :], xT[:c, :])
                                negsq = lwork.tile([1, N], f32, tag="negsq",
                                                   bufs=1)
                                negsq_dst = negsq[:, :]
                        else:
                            lhs_sc = lwork.tile([c, N], f32, tag="lhs_sc",
                                                bufs=1)
                            rhs_sc = xT
                            nc.vector.tensor_scalar_mul(lhs_sc[:c, :],
                                                        xT[:c, :], 2.0)
                            negsq = lwork.tile([1, N], f32, tag="negsq", bufs=1)
                            negsq_dst = negsq[:, :]
                        for ch in range(4):
                            cs = slice(ch * 512, (ch + 1) * 512)
                            nps = pp.tile([1, 512], f32, tag="negsq_ps")
                            nc.tensor.matmul(nps, lhsT=onesC[:c, :],
                                             rhs=sq[:c, cs],
                                             start=True, stop=True)
                            nc.scalar.mul(negsq_dst[:, cs], nps, -1.0)
                        if aug and not aligned:
                            nc.sync.dma_start(out=rhs_sc[c:c + 1, :],
                                              in_=negsq[:, :])

                    # --- per-block pipeline
                    xout = [persist.tile([128, N], f32, tag=f"{out_tag}_{t}",
                                         name=f"{out_tag}_{t}")
                            for t in range(notile)]
                    with tc.tile_pool(name=f"sc_ps{l}", bufs=1,
                                      space="PSUM") as sp, \
                         tc.tile_pool(name=f"uv_ps{l}", bufs=1,
                                      space="PSUM") as up, \
                         tc.tile_pool(name=f"tr_ps{l}", bufs=2,
                                      space="PSUM") as tp:
                        for b in range(NBLK):
                            bsl = slice(b * 128, (b + 1) * 128)
                            # uT/vT (node-major, bn-scaled weights)
                            uT_ps = up.tile([128, o], f32, tag="uT_ps")
                            nc.tensor.matmul(uT_ps, lhsT=xT[:c, bsl],
                                             rhs=WaT[l][:c, :],
                                             start=True, stop=True)
                            uT_sb = lwork.tile([128, o], f32, tag="uT_sb")
                            nc.scalar.copy(uT_sb, uT_ps)
                            nc.sync.dma_start(
                                out=bass.AP(tensor=uT_d[l], offset=b * 128 * o,
                                            ap=[[o, 128], [1, o]]),
                                in_=uT_sb)
                            vT_ps = up.tile([128, o], f32, tag="vT_ps")
                            nc.tensor.matmul(vT_ps, lhsT=xT[:c, bsl],
                                             rhs=WdT[l][:c, :],
                                             start=True, stop=True)
                            vT_sb = lwork.tile([128, o], f32, tag="vT_sb")
                            nc.scalar.copy(vT_sb, vT_ps)

                            # scores
                            scps = sp.tile([128, N], f32, tag="scps")
                            for ch in range(4):
                                cs = slice(ch * 512, (ch + 1) * 512)
                                if aug:
                                    nc.tensor.matmul(scps[:, cs],
                                                     lhsT=lhs_sc[:c + 1, bsl],
                                                     rhs=rhs_sc[:c + 1, cs],
                                                     start=True, stop=True)
                                else:
                                    nc.tensor.matmul(scps[:, cs],
                                                     lhsT=lhs_sc[:c, bsl],
                                                     rhs=rhs_sc[:c, cs],
                                                     start=True, stop=False)
                                    nc.tensor.matmul(scps[:, cs], lhsT=ones1,
                                                     rhs=negsq[:, cs],
                                                     start=False, stop=True)
                            sc = lwork.tile([128, N], f32, tag="sc")
                            nc.scalar.copy(sc, scps)

                            # exact top-20 (self included): 3 rounds of max8
                            idxb = lwork.tile([128, KNN], u16, tag="idxb")
                            vals = lwork.tile([128, 8], f32, tag="vals")
                            idx3 = lwork.tile([128, 8], u16, tag="idx3")
                            nc.vector.max(vals, sc)
                            nc.vector.max_index(idxb[:, 0:8], vals, sc)
                            nc.vector.match_replace(sc, vals, sc, NEG_BIG)
                            nc.vector.max(vals, sc)
                            nc.vector.max_index(idxb[:, 8:16], vals, sc)
                            nc.vector.match_replace(sc, vals, sc, NEG_BIG)
                            nc.vector.max(vals, sc)
                            nc.vector.max_index(idx3, vals, sc)
                            nc.vector.tensor_copy(idxb[:, 16:20], idx3[:, 0:4])

                            # n-major contiguous store: list[n*20+k] = idxb[n,k]
                            dst1 = bass.AP(tensor=list_d[l], offset=b * 2560,
                                           ap=[[KNN, 128], [1, KNN]])
                            nc.sync.dma_start(out=dst1, in_=idxb[:, :])
                            # j-major 16-wrap read for dma_gather:
                            # wrapped[16g+q, j*8+r] = list[(16r+q)*20 + j]
                            wrapped = lwork.tile([128, 160], u16, tag="wrapped")
                            for g in range(8):
                                src2 = bass.AP(tensor=list_d[l],
                                               offset=b * 2560,
                                               ap=[[20, 16], [1, 20], [320, 8]])
                                nc.sync.dma_start(
                                    out=wrapped[g * 16:(g + 1) * 16, :],
                                    in_=src2)

                            # gather 20 neighbor uT rows per node from DRAM
                            gath = lwork.tile([128, KNN * o], f32, tag="gath")
                            src_rows = bass.AP(tensor=uT_d[l], offset=0,
                                               ap=[[o, N], [1, o]])
                            nc.gpsimd.dma_gather(
                                out_ap=gath.rearrange("p (k o) -> p k o", o=o),
                                in_ap=src_rows,
                                idxs_ap=wrapped[:, :].bitcast(i16),
                                num_idxs=2560,
                                num_idxs_reg=2560,
                                elem_size=o)

                            # k-max on GPSIMD, then +v +bias, lrelu on DVE
                            m_sb = lwork.tile([128, o], f32, tag="m_sb")
                            nc.vector.tensor_reduce(
                                m_sb,
                                gath.rearrange("p (k o) -> p o k", o=o),
                                axis=mybir.AxisListType.X, op=Alu.max)
                            z = lwork.tile([128, o], f32, tag="z")
                            nc.vector.tensor_add(z, m_sb, vT_sb)
                            zb = lwork.tile([128, o], f32, tag="zb")
                            nc.vector.tensor_add(zb, z, bias_bc[l])
                            y = lwork.tile([128, o], f32, tag="yb")
                            nc.vector.scalar_tensor_tensor(
                                y, zb, NEG_SLOPE, zb,
                                op0=Alu.mult, op1=Alu.max)

                            # transpose back to [channel, node] for next layer
                            for t in range(notile):
                                op = min(128, o - t * 128)
                                yt_ps = tp.tile([128, 128], f32, tag="yt_ps")
                                nc.tensor.transpose(
                                    yt_ps[:op, :],
                                    y[:, t * 128:t * 128 + op],
                                    ident)
                                nc.scalar.copy(xout[t][:op, bsl],
                                               yt_ps[:op, :])
                return xout

            x1 = edge_conv(0, 3, 64, x0T, "x1")
            x2 = edge_conv(1, 64, 64, x1[0], "x2")
            x3 = edge_conv(2, 64, 128, x2[0], "x3")
            x4 = edge_conv(3, 128, 256, x3[0], "x4")

            # ---- final 1x1 conv (W5, bf16) + BN + lrelu + global max/avg pool
            xcat_parts = [(x1[0], 64), (x2[0], 64), (x3[0], 128),
                          (x4[0], 128), (x4[1], 128)]
            inv_n = 1.0 / float(N)
            with tc.tile_pool(name="f_ps", bufs=4, space="PSUM") as fp, \
                 tc.tile_pool(name="fwork", bufs=2) as fw, \
                 tc.tile_pool(name="fb16", bufs=1) as fb:
                xcb = []
                for i, (xp, ck) in enumerate(xcat_parts):
                    xtile = fb.tile([max(ck, 16), N], bf16, tag=f"xcb{i}",
                                    name=f"xcb{i}")
                    nc.vector.tensor_copy(xtile[:ck, :], xp[:ck, :])
                    xcb.append(xtile)
                for ot in range(8):
                    osl = slice(ot * 128, (ot + 1) * 128)
                    sums = fw.tile([128, 4], f32, tag="sums")
                    gmax = fw.tile([128, 512], f32, tag="gmax512")
                    for chn in range(4):
                        cs = slice(chn * 512, (chn + 1) * 512)
                        fps = fp.tile([128, 512], f32, tag="fps")
                        for i, (xp, ck) in enumerate(xcat_parts):
                            nc.tensor.matmul(fps, lhsT=W5T[i][:ck, osl],
                                             rhs=xcb[i][:ck, cs],
                                             start=(i == 0), stop=(i == 4))
                        ybn = fw.tile([128, 512], f32, tag="fybn")
                        nc.scalar.activation(ybn, fps, Act.Identity,
                                             bias=bn5_b[ot], scale=bn5_s[ot])
                        feat = fw.tile([128, 512], f32, tag="feat")
                        nc.vector.scalar_tensor_tensor(
                            feat, ybn, NEG_SLOPE, ybn,
                            op0=Alu.mult, op1=Alu.max,
                            accum_out=sums[:, chn:chn + 1])
                        if chn == 0:
                            nc.vector.tensor_copy(gmax, feat)
                        else:
                            nc.vector.tensor_max(gmax, gmax, feat)
                    gm = fw.tile([128, 1], f32, tag="gm")
                    nc.vector.tensor_reduce(gm, gmax, axis=mybir.AxisListType.X,
                                            op=Alu.max)
                    ga = fw.tile([128, 1], f32, tag="ga")
                    nc.vector.tensor_reduce(ga, sums, axis=mybir.AxisListType.X,
                                            op=Alu.add)
                    nc.vector.tensor_scalar_mul(ga, ga, inv_n)
                    nc.sync.dma_start(
                        out=bass.AP(tensor=out_d, offset=ot * 128,
                                    ap=[[1, 128]]),
                        in_=gm[:, :])
                    nc.sync.dma_start(
                        out=bass.AP(tensor=out_d, offset=1024 + ot * 128,
                                    ap=[[1, 128]]),
                        in_=ga[:, :])


def host_prepare(inputs):
    """Full inputs -> per-core input maps (host-side layout transforms)."""
    x = np.asarray(inputs["x"], dtype=np.float32)
    B = x.shape[0]
    shared = {}
    for l, (c, o) in enumerate(LAYERS):
        W = np.asarray(inputs[f"W{l+1}"], dtype=np.float32)
        Wa = W[:, :c]
        Wd = W[:, c:] - Wa
        bn = np.asarray(inputs[f"bn{l+1}"], dtype=np.float32)
        g, b, m, v = bn[0], bn[1], bn[2], bn[3]
        s = g / np.sqrt(v + BN_EPS)
        shared[f"WaT{l+1}"] = np.ascontiguousarray((Wa * s[:, None]).T)
        shared[f"WdT{l+1}"] = np.ascontiguousarray((Wd * s[:, None]).T)
        shared[f"biasv{l+1}"] = np.ascontiguousarray(b - m * s)
    W5 = np.asarray(inputs["W5"], dtype=np.float32)
    for lo, hi in W5_CHUNKS:
        shared[f"W5T_{lo}"] = np.ascontiguousarray(W5[:, lo:hi].T)
    shared["bn5T"] = np.ascontiguousarray(
        np.asarray(inputs["bn5"], dtype=np.float32).T)
    shared["ident"] = np.eye(128, dtype=np.float32)
    return [dict(shared, xT=np.ascontiguousarray(x[b].T)) for b in range(B)]


_CACHED = {}


def _get_compiled():
    if "nc" not in _CACHED:
        nc = bacc.Bacc("TRN2", target_bir_lowering=False, debug=False,
                       num_devices=8)
        build_dgcnn(nc)
        nc.compile()
        _CACHED["nc"] = nc
    return _CACHED["nc"]


def kernel(**inputs):
    from concourse.bass_utils import run_bass_kernel_spmd
    nc = _get_compiled()
    in_maps = host_prepare(inputs)
    res = run_bass_kernel_spmd(nc, in_maps, list(range(len(in_maps))))
    out = np.stack([np.asarray(res.results[b]["out"]).reshape(-1)
                    for b in range(len(in_maps))], axis=0)
    return out.astype(np.float32)


# revision 11
# speedup vs baseline: 1.2523x; 1.0469x over previous
"""DGCNN encoder Bass kernel for Trainium2, data-parallel over batch on 8 cores.

Per core (one sample, x: (2048, 3)):
  4 EdgeConv layers + final 1x1 conv + global max/avg pool -> (2048,) output row.

Restructure (exact): edgeconv(x)[n] = lrelu( max_{j in knn(n)} (s*Wa @ x_j)
  + (s*(Wb-Wa)) @ x_n + bias ) with the positive BN scale s folded into the
weights host-side (max commutes with positive scaling).

v2 layout: kNN selection unchanged (exact fp32 max8/max_index/match_replace
rounds on DVE). The neighbor aggregation is restructured as a DRAM row-gather:
uT rows (node-major, bn-scaled) are written per block to DRAM, then one
dma_gather per block fetches the 20 neighbor rows of each node into
[128 nodes, 20, o]; the k-max reduce runs on GPSIMD, bias+lrelu on DVE, and a
PE transpose brings the result back to the [channel, node] layout the next
layer consumes. All plain DMAs are issued from the SP/ACT engines (HWDGE)
instead of GPSIMD to avoid the ~1us SWDGE descriptor-generation tax per call.
"""
import sys
sys.path.insert(0, '/opt/trn_rl_repo')

import numpy as np
import concourse.bass as bass
import concourse.bacc as bacc
import concourse.tile as tile
from concourse import mybir

f32 = mybir.dt.float32
u16 = mybir.dt.uint16
i16 = mybir.dt.int16
bf16 = mybir.dt.bfloat16
Alu = mybir.AluOpType
Act = mybir.ActivationFunctionType

N = 2048
NBLK = N // 128
KNN = 20
NEG_SLOPE = 0.2
BN_EPS = 1e-5
NEG_BIG = -1e30

DEBUG_DUMPS = False

# (C_in, O) per edge-conv layer
LAYERS = [(3, 64), (64, 64), (64, 128), (128, 256)]
# W5^T host-side chunks aligned to the xcat source tiles
W5_CHUNKS = [(0, 64), (64, 128), (128, 256), (256, 384), (384, 512)]


def _ceil(a, b):
    return (a + b - 1) // b


def build_dgcnn(nc):
    """Emit the full per-core DGCNN program into nc."""
    xT_d = nc.dram_tensor("xT", [3, N], f32, kind="ExternalInput")
    WaT_d = [nc.dram_tensor(f"WaT{l+1}", [c, o], f32, kind="ExternalInput")
             for l, (c, o) in enumerate(LAYERS)]
    WdT_d = [nc.dram_tensor(f"WdT{l+1}", [c, o], f32, kind="ExternalInput")
             for l, (c, o) in enumerate(LAYERS)]
    bias_d = [nc.dram_tensor(f"biasv{l+1}", [o], f32, kind="ExternalInput")
              for l, (c, o) in enumerate(LAYERS)]
    W5T_d = [nc.dram_tensor(f"W5T_{lo}", [hi - lo, 1024], f32,
                            kind="ExternalInput") for lo, hi in W5_CHUNKS]
    bn5T_d = nc.dram_tensor("bn5T", [1024, 4], f32, kind="ExternalInput")
    ident_d = nc.dram_tensor("ident", [128, 128], f32, kind="ExternalInput")
    out_d = nc.dram_tensor("out", [2048], f32, kind="ExternalOutput")
    # per-layer DRAM bounces: j-major neighbor index list and node-major uT rows
    list_d = [nc.dram_tensor(f"idxlist{l}", [NBLK * 128 * KNN], u16,
                             kind="Internal") for l in range(4)]
    uT_d = [nc.dram_tensor(f"uT{l}", [N * o], f32, kind="Internal")
            for l, (c, o) in enumerate(LAYERS)]
    if DEBUG_DUMPS:
        dbg_wrap = nc.dram_tensor("dbg_wrap", [128 * 160], u16,
                                  kind="Internal")
        dbg_gath = nc.dram_tensor("dbg_gath", [128 * KNN * 64], f32,
                                  kind="Internal")
        dbg_y = nc.dram_tensor("dbg_y", [128 * 64], f32, kind="Internal")

    with tile.TileContext(nc) as tc:
        from contextlib import ExitStack
        ctx = ExitStack()
        with ctx:
            persist = ctx.enter_context(tc.tile_pool(name="persist", bufs=1))
            work = ctx.enter_context(tc.tile_pool(name="work", bufs=2))

            onesC = persist.tile([128, 1], f32, tag="onesC")
            nc.vector.memset(onesC, 1.0)
            ones1 = persist.tile([1, 128], f32, tag="ones1")
            nc.vector.memset(ones1, 1.0)
            eps_t = persist.tile([128, 1], f32, tag="eps")
            nc.vector.memset(eps_t, BN_EPS)
            ident = persist.tile([128, 128], f32, tag="ident")
            nc.sync.dma_start(out=ident, in_=ident_d[:, :])

            # ---- bn5 param prep: (C, 4) rows [gamma, beta, mean, var] ->
            #      scale (C,1), bias (C,1) tiles per 128-channel chunk
            def prep_bn(bn_dram, channels, name):
                scales, biases = [], []
                for t in range(_ceil(channels, 128)):
                    p = min(128, channels - t * 128)
                    raw = work.tile([128, 4], f32, tag="bnraw")
                    src = bass.AP(tensor=bn_dram, offset=t * 128 * 4,
                                  ap=[[4, p], [1, 4]])
                    nc.sync.dma_start(out=raw[:p, :], in_=src)
                    s_t = persist.tile([128, 1], f32, tag=f"{name}_s{t}")
                    b_t = persist.tile([128, 1], f32, tag=f"{name}_b{t}")
                    tmp = work.tile([128, 1], f32, tag="bntmp")
                    nc.scalar.activation(tmp[:p], raw[:p, 3:4], Act.Sqrt,
                                         bias=eps_t[:p], scale=1.0)
                    nc.vector.reciprocal(tmp[:p], tmp[:p])
                    nc.vector.tensor_mul(s_t[:p], raw[:p, 0:1], tmp[:p])
                    nc.vector.tensor_mul(tmp[:p], raw[:p, 2:3], s_t[:p])
                    nc.vector.tensor_sub(b_t[:p], raw[:p, 1:2], tmp[:p])
                    scales.append(s_t)
                    biases.append(b_t)
                return scales, biases

            bn5_s, bn5_b = prep_bn(bn5T_d, 1024, "bn5")

            # ---- weights (transposed + bn-scaled host-side)
            WaT, WdT, bias_bc = [], [], []
            for l, (c, o) in enumerate(LAYERS):
                wa = persist.tile([max(c, 16), o], f32, tag=f"WaT{l}",
                                  name=f"WaT{l}")
                nc.sync.dma_start(out=wa[:c, :], in_=WaT_d[l][:, :])
                wd = persist.tile([max(c, 16), o], f32, tag=f"WdT{l}",
                                  name=f"WdT{l}")
                nc.sync.dma_start(out=wd[:c, :], in_=WdT_d[l][:, :])
                WaT.append(wa)
                WdT.append(wd)
                # bias broadcast tile [128, o] (replicated rows)
                bb = persist.tile([128, o], f32, tag=f"biasbc{l}",
                                  name=f"biasbc{l}")
                nc.sync.dma_start(
                    out=bb,
                    in_=bass.AP(tensor=bias_d[l], offset=0,
                                ap=[[0, 128], [1, o]]))
                bias_bc.append(bb)
            W5T = []
            for i, (lo, hi) in enumerate(W5_CHUNKS):
                t5 = persist.tile([max(hi - lo, 16), 1024], bf16,
                                  tag=f"W5T_{lo}", name=f"W5T_{lo}")
                nc.gpsimd.dma_start(out=t5[:hi - lo, :], in_=W5T_d[i][:, :])
                W5T.append(t5)

            x0T = persist.tile([16, N], f32, tag="x0T")
            nc.sync.dma_start(out=x0T[:3, :], in_=xT_d[:, :])

            # ---- edge conv layers
            def edge_conv(l, c, o, xT, out_tag):
                notile = _ceil(o, 128)
                aug = c + 1 <= 65
                with ExitStack() as lx:
                    lwork = lx.enter_context(
                        tc.tile_pool(name=f"lwork{l}", bufs=2))
                    prep_ps = tc.tile_pool(name=f"prep_ps{l}", bufs=2,
                                           space="PSUM")
                    with prep_ps as pp:
                        sq = lwork.tile([max(c, 16), N], f32, tag="sq", bufs=1)
                        nc.scalar.square(sq[:c, :], xT[:c, :])
                        if aug:
                            lhs_sc = lwork.tile([c + 1, N], f32, tag="lhs_sc",
                                                bufs=1)
                            rhs_sc = lwork.tile([c + 1, N], f32, tag="rhs_sc",
                                                bufs=1)
                            aligned = (c % 32) == 0
                            if aligned:
                                nc.vector.tensor_scalar_mul(lhs_sc[:c, :],
                                                            xT[:c, :], 2.0)
                                nc.vector.memset(lhs_sc[c:c + 1, :], 1.0)
                                nc.scalar.copy(rhs_sc[:c, :], xT[:c, :])
                                negsq_dst = rhs_sc[c:c + 1, :]
                            else:
                                # engine APs must start 32-aligned: fill the
                                # ones row via full-height memset; negsq goes
                                # through a base-0 tile + contiguous SBUF DMA
                                nc.vector.memset(lhs_sc[:c + 1, :], 1.0)
                                nc.vector.tensor_scalar_mul(lhs_sc[:c, :],
                                                            xT[:c, :], 2.0)
                                nc.scalar.copy(rhs_sc[:c, :], xT[:c, :])
                                negsq = lwork.tile([1, N], f32, tag="negsq",
                                                   bufs=1)
                                negsq_dst = negsq[:, :]
                        else:
                            lhs_sc = lwork.tile([c, N], f32, tag="lhs_sc",
                                                bufs=1)
                            rhs_sc = xT
                            nc.vector.tensor_scalar_mul(lhs_sc[:c, :],
                                                        xT[:c, :], 2.0)
                            negsq = lwork.tile([1, N], f32, tag="negsq", bufs=1)
                            negsq_dst = negsq[:, :]
                        for ch in range(4):
                            cs = slice(ch * 512, (ch + 1) * 512)
                            nps = pp.tile([1, 512], f32, tag="negsq_ps")
                            nc.tensor.matmul(nps, lhsT=onesC[:c, :],
                                             rhs=sq[:c, cs],
                                             start=True, stop=True)
                            nc.scalar.mul(negsq_dst[:, cs], nps, -1.0)
                        if aug and not aligned:
                            nc.sync.dma_start(out=rhs_sc[c:c + 1, :],
                                              in_=negsq[:, :])

                    # --- per-block pipeline
                    xout = [persist.tile([128, N], f32, tag=f"{out_tag}_{t}",
                                         name=f"{out_tag}_{t}")
                            for t in range(notile)]
                    with tc.tile_pool(name=f"sc_ps{l}", bufs=1,
                                      space="PSUM") as sp, \
                         tc.tile_pool(name=f"uv_ps{l}", bufs=1,
                                      space="PSUM") as up, \
                         tc.tile_pool(name=f"tr_ps{l}", bufs=2,
                                      space="PSUM") as tp:
                        # phase 1: uT/vT for ALL blocks (gathers read the
                        # whole uT table, so every row must land first)
                        vT_all = []
                        for b in range(NBLK):
                            bsl = slice(b * 128, (b + 1) * 128)
                            uT_ps = up.tile([128, o], f32, tag="uT_ps")
                            nc.tensor.matmul(uT_ps, lhsT=xT[:c, bsl],
                                             rhs=WaT[l][:c, :],
                                             start=True, stop=True)
                            uT_sb = lwork.tile([128, o], f32, tag="uT_sb")
                            nc.scalar.copy(uT_sb, uT_ps)
                            nc.sync.dma_start(
                                out=bass.AP(tensor=uT_d[l], offset=b * 128 * o,
                                            ap=[[o, 128], [1, o]]),
                                in_=uT_sb)
                            vT_ps = up.tile([128, o], f32, tag="vT_ps")
                            nc.tensor.matmul(vT_ps, lhsT=xT[:c, bsl],
                                             rhs=WdT[l][:c, :],
                                             start=True, stop=True)
                            vT_sb = lwork.tile([128, o], f32, tag="vT_sb",
                                               bufs=NBLK)
                            nc.scalar.copy(vT_sb, vT_ps)
                            vT_all.append(vT_sb)

                        # phase 2: per-block score->top20->gather, with the
                        # post-gather tail software-pipelined one block behind
                        # so DVE selection overlaps the previous block's DMA
                        pending = []

                        def finish(b, gath, vT_sb):
                            bsl = slice(b * 128, (b + 1) * 128)
                            m_sb = lwork.tile([128, o], f32, tag="m_sb")
                            nc.vector.tensor_reduce(
                                m_sb,
                                gath.rearrange("p (k o) -> p o k", o=o),
                                axis=mybir.AxisListType.X, op=Alu.max)
                            z = lwork.tile([128, o], f32, tag="z")
                            nc.vector.tensor_add(z, m_sb, vT_sb)
                            zb = lwork.tile([128, o], f32, tag="zb")
                            nc.vector.tensor_add(zb, z, bias_bc[l])
                            y = lwork.tile([128, o], f32, tag="yb")
                            nc.vector.scalar_tensor_tensor(
                                y, zb, NEG_SLOPE, zb,
                                op0=Alu.mult, op1=Alu.max)
                            if DEBUG_DUMPS and l == 0 and b == 0:
                                nc.sync.dma_start(
                                    out=bass.AP(tensor=dbg_y, offset=0,
                                                ap=[[o, 128], [1, o]]),
                                    in_=y[:, :])
                            for t in range(notile):
                                op = min(128, o - t * 128)
                                yt_ps = tp.tile([128, 128], f32, tag="yt_ps")
                                nc.tensor.transpose(
                                    yt_ps[:op, :],
                                    y[:, t * 128:t * 128 + op],
                                    ident)
                                nc.scalar.copy(xout[t][:op, bsl],
                                               yt_ps[:op, :])

                        for b in range(NBLK):
                            bsl = slice(b * 128, (b + 1) * 128)
                            # scores
                            scps = sp.tile([128, N], f32, tag="scps")
                            for ch in range(4):
                                cs = slice(ch * 512, (ch + 1) * 512)
                                if aug:
                                    nc.tensor.matmul(scps[:, cs],
                                                     lhsT=lhs_sc[:c + 1, bsl],
                                                     rhs=rhs_sc[:c + 1, cs],
                                                     start=True, stop=True)
                                else:
                                    nc.tensor.matmul(scps[:, cs],
                                                     lhsT=lhs_sc[:c, bsl],
                                                     rhs=rhs_sc[:c, cs],
                                                     start=True, stop=False)
                                    nc.tensor.matmul(scps[:, cs], lhsT=ones1,
                                                     rhs=negsq[:, cs],
                                                     start=False, stop=True)
                            sc = lwork.tile([128, N], f32, tag="sc")
                            nc.scalar.copy(sc, scps)

                            # exact top-20 (self included): 3 rounds of max8
                            idxb = lwork.tile([128, KNN], u16, tag="idxb")
                            vals = lwork.tile([128, 8], f32, tag="vals")
                            idx3 = lwork.tile([128, 8], u16, tag="idx3")
                            nc.vector.max(vals, sc)
                            nc.vector.max_index(idxb[:, 0:8], vals, sc)
                            nc.vector.match_replace(sc, vals, sc, NEG_BIG)
                            nc.vector.max(vals, sc)
                            nc.vector.max_index(idxb[:, 8:16], vals, sc)
                            nc.vector.match_replace(sc, vals, sc, NEG_BIG)
                            nc.vector.max(vals, sc)
                            nc.vector.max_index(idx3, vals, sc)
                            nc.vector.tensor_copy(idxb[:, 16:20], idx3[:, 0:4])

                            # n-major contiguous store: list[n*20+k] = idxb[n,k]
                            dst1 = bass.AP(tensor=list_d[l], offset=b * 2560,
                                           ap=[[KNN, 128], [1, KNN]])
                            nc.sync.dma_start(out=dst1, in_=idxb[:, :])
                            # j-major 16-wrap read for dma_gather:
                            # wrapped[16g+q, j*8+r] = list[(16r+q)*20 + j]
                            wrapped = lwork.tile([128, 160], u16, tag="wrapped")
                            for g in range(8):
                                src2 = bass.AP(tensor=list_d[l],
                                               offset=b * 2560,
                                               ap=[[20, 16], [1, 20], [320, 8]])
                                nc.sync.dma_start(
                                    out=wrapped[g * 16:(g + 1) * 16, :],
                                    in_=src2)

                            # gather 20 neighbor uT rows per node from DRAM
                            gath = lwork.tile([128, KNN * o], f32, tag="gath",
                                              bufs=3)
                            src_rows = bass.AP(tensor=uT_d[l], offset=0,
                                               ap=[[o, N], [1, o]])
                            nc.gpsimd.dma_gather(
                                out_ap=gath.rearrange("p (k o) -> p k o", o=o),
                                in_ap=src_rows,
                                idxs_ap=wrapped[:, :].bitcast(i16),
                                num_idxs=2560,
                                num_idxs_reg=2560,
                                elem_size=o)

                            if DEBUG_DUMPS and l == 0 and b == 0:
                                nc.sync.dma_start(
                                    out=bass.AP(tensor=dbg_wrap, offset=0,
                                                ap=[[160, 128], [1, 160]]),
                                    in_=wrapped[:, :])
                                nc.sync.dma_start(
                                    out=bass.AP(tensor=dbg_gath, offset=0,
                                                ap=[[KNN * o, 128],
                                                    [1, KNN * o]]),
                                    in_=gath[:, :])

                            pending.append((b, gath, vT_all[b]))
                            if len(pending) > 1:
                                finish(*pending.pop(0))
                        while pending:
                            finish(*pending.pop(0))
                return xout

            x1 = edge_conv(0, 3, 64, x0T, "x1")
            x2 = edge_conv(1, 64, 64, x1[0], "x2")
            x3 = edge_conv(2, 64, 128, x2[0], "x3")
            x4 = edge_conv(3, 128, 256, x3[0], "x4")

            # ---- final 1x1 conv (W5, bf16) + BN + lrelu + global max/avg pool
            xcat_parts = [(x1[0], 64), (x2[0], 64), (x3[0], 128),
                          (x4[0], 128), (x4[1], 128)]
            inv_n = 1.0 / float(N)
            with tc.tile_pool(name="f_ps", bufs=4, space="PSUM") as fp, \
                 tc.tile_pool(name="fwork", bufs=2) as fw, \
                 tc.tile_pool(name="fb16", bufs=1) as fb:
                xcb = []
                for i, (xp, ck) in enumerate(xcat_parts):
                    xtile = fb.tile([max(ck, 16), N], bf16, tag=f"xcb{i}",
                                    name=f"xcb{i}")
                    nc.vector.tensor_copy(xtile[:ck, :], xp[:ck, :])
                    xcb.append(xtile)
                for ot in range(8):
                    osl = slice(ot * 128, (ot + 1) * 128)
                    sums = fw.tile([128, 4], f32, tag="sums")
                    gmax = fw.tile([128, 512], f32, tag="gmax512")
                    for chn in range(4):
                        cs = slice(chn * 512, (chn + 1) * 512)
                        fps = fp.tile([128, 512], f32, tag="fps")
                        for i, (xp, ck) in enumerate(xcat_parts):
                            nc.tensor.matmul(fps, lhsT=W5T[i][:ck, osl],
                                             rhs=xcb[i][:ck, cs],
                                             start=(i == 0), stop=(i == 4))
                        ybn = fw.tile([128, 512], f32, tag="fybn")
                        nc.scalar.activation(ybn, fps, Act.Identity,
                                             bias=bn5_b[ot], scale=bn5_s[ot])
                        feat = fw.tile([128, 512], f32, tag="feat")
                        nc.vector.scalar_tensor_tensor(
                            feat, ybn, NEG_SLOPE, ybn,
                            op0=Alu.mult, op1=Alu.max,
                            accum_out=sums[:, chn:chn + 1])
                        if chn == 0:
                            nc.vector.tensor_copy(gmax, feat)
                        else:
                            nc.vector.tensor_max(gmax, gmax, feat)
                    gm = fw.tile([128, 1], f32, tag="gm")
                    nc.vector.tensor_reduce(gm, gmax, axis=mybir.AxisListType.X,
                                            op=Alu.max)
                    ga = fw.tile([128, 1], f32, tag="ga")
                    nc.vector.tensor_reduce(ga, sums, axis=mybir.AxisListType.X,
                                            op=Alu.add)
                    nc.vector.tensor_scalar_mul(ga, ga, inv_n)
                    nc.sync.dma_start(
                        out=bass.AP(tensor=out_d, offset=ot * 128,
                                    ap=[[1, 128]]),
                        in_=gm[:, :])
                    nc.sync.dma_start(
                        out=bass.AP(tensor=out_d, offset=1024 + ot * 128,
                                    ap=[[1, 128]]),
                        in_=ga[:, :])


def host_prepare(inputs):
    """Full inputs -> per-core input maps (host-side layout transforms)."""
    x = np.asarray(inputs["x"], dtype=np.float32)
    B = x.shape[0]
    shared = {}
    for l, (c, o) in enumerate(LAYERS):
        W = np.asarray(inputs[f"W{l+1}"], dtype=np.float32)
        Wa = W[:, :c]
        Wd = W[:, c:] - Wa
        bn = np.asarray(inputs[f"bn{l+1}"], dtype=np.float32)
        g, b, m, v = bn[0], bn[1], bn[2], bn[3]
        s = g / np.sqrt(v + BN_EPS)
        shared[f"WaT{l+1}"] = np.ascontiguousarray((Wa * s[:, None]).T)
        shared[f"WdT{l+1}"] = np.ascontiguousarray((Wd * s[:, None]).T)
        shared[f"biasv{l+1}"] = np.ascontiguousarray(b - m * s)
    W5 = np.asarray(inputs["W5"], dtype=np.float32)
    for lo, hi in W5_CHUNKS:
        shared[f"W5T_{lo}"] = np.ascontiguousarray(W5[:, lo:hi].T)
    shared["bn5T"] = np.ascontiguousarray(
        np.asarray(inputs["bn5"], dtype=np.float32).T)
    shared["ident"] = np.eye(128, dtype=np.float32)
    return [dict(shared, xT=np.ascontiguousarray(x[b].T)) for b in range(B)]


_CACHED = {}


def _get_compiled():
    if "nc" not in _CACHED:
        nc = bacc.Bacc("TRN2", target_bir_lowering=False, debug=False,
                       num_devices=8)
        build_dgcnn(nc)
        nc.compile()
        _CACHED["nc"] = nc
    return _CACHED["nc"]


def kernel(**inputs):
    from concourse.bass_utils import run_bass_kernel_spmd
    nc = _get_compiled()
    in_maps = host_prepare(inputs)
    res = run_bass_kernel_spmd(nc, in_maps, list(range(len(in_maps))))
    out = np.stack([np.asarray(res.results[b]["out"]).reshape(-1)
                    for b in range(len(in_maps))], axis=0)
    return out.astype(np.float32)
